# revision 1
# baseline (speedup 1.0000x reference)
"""Trainium2 Bass kernel for sorted segment_max (ClusterPool).

Problem: features [2M, 128] f32, segment_ids [2M] sorted int, num_clusters
10000 -> output [10000, 128] = per-cluster max over rows.

Strategy (8 NeuronCores, SPMD):
  - Shard rows: core c gets rows [c*250k, (c+1)*250k). Sorted ids => each
    core covers a contiguous cluster range (~1252 clusters), padded to 1280
    local clusters = 10 batches x 128.
  - Host precomputes, per core, per batch of 128 clusters, an int16 gather
    index table: cluster p's rows (padded by repeating its first row to a
    fixed slot count L). Indices are relative to a per-batch 32768-row
    window of the core's feature slice (clusters are contiguous, so every
    batch's rows fit in one window; verified at runtime).
  - Device: per batch, dma_gather rows into SBUF [128, L, 128] (cluster p on
    partition p), tensor_reduce max over the slot axis -> [128, 128], store
    to the partial output rows. Two gathers of L/2 slots per batch to halve
    SBUF footprint.
  - Host combines the 8 partial outputs (adjacent cores overlap in at most
    one boundary cluster) with np.maximum.
"""

import os
import sys

import numpy as np

sys.path.insert(0, "/opt/trn_rl_repo")

N_POINTS = 2_000_000
D = 128
N_CLUSTERS = 10_000
N_CORES = 8
RPC = N_POINTS // N_CORES  # rows per core
NCL = 1280  # padded local clusters per core
NBATCH = NCL // 128
WINDOW = 32768  # dma_gather int16 index window (rows)

_last_results = None  # BassKernelResults of the most recent run (for test.py)


def _apply_drain_patch():
    """walrus TPB_CTRL supports a single sync wait; TileContext's tail drain
    accumulates one wait per outstanding proc.  Split them across NOPs."""
    import concourse.mybir as mybir
    import concourse.tile as tile
    from concourse.vector_clock import ScopedClock

    if getattr(tile.TileContext, "_drain_patched", False):
        return

    def _patched(self, tick_clock, wait_clock):
        nc = self.nc
        nop = nc.sync.nop(nofuse=True, hint="tail_drain_waits")
        wait_clock.add_sem_waits(nop.ins, ScopedClock({None: tick_clock.global_clock}))
        si = nop.ins.sync_info
        waits = list(si.on_wait) if si is not None and si.on_wait else []
        if len(waits) > 1:
            si.on_wait = waits[:1]
            for i in range(1, len(waits)):
                extra = nc.sync.nop(nofuse=True, hint=f"tail_drain_waits_{i}")
                if extra.ins.sync_info is None:
                    extra.ins.sync_info = mybir.SyncInfo(
                        on_wait=waits[i : i + 1], on_update=[]
                    )
                else:
                    extra.ins.sync_info.on_wait = waits[i : i + 1]
        nc.sync.drain()
        nc.all_engine_barrier()
        assert self.sems is not None
        popped = nc._tile_sem_poison_stack.pop()
        assert popped is self._sem_poison
        nc.clear_and_free_semaphores(list(self.sems.allocated().values()))
        nc.all_engine_barrier()

    tile.TileContext._drain_and_barrier = _patched
    tile.TileContext._drain_patched = True


def _build_program(LH, windows):
    """Build the SPMD Bass program.  LH = slots per half-gather; windows =
    per-batch window base rows (compile-time constants, shared by all cores)."""
    import concourse.bacc as bacc
    import concourse.mybir as mybir
    import concourse.tile as tile

    _apply_drain_patch()

    NG = LH // 8  # gathers per half-batch (1024 idx each: 8 slots/partition)
    IW = 64  # idx cols per gather (1024/16)

    nc = bacc.Bacc(None, num_swdge_queues=4)
    f_in = nc.dram_tensor("features", [RPC, D], mybir.dt.float32, kind="ExternalInput")
    i_in = nc.dram_tensor(
        "gidx", [NBATCH, 2, 128, NG * IW], mybir.dt.int16, kind="ExternalInput"
    )
    p_out = nc.dram_tensor(
        "partial", [NCL, D], mybir.dt.float32, kind="ExternalOutput"
    )

    with tile.TileContext(nc) as tc:
        with (
            tc.tile_pool(name="gp", bufs=2) as gp,
            tc.tile_pool(name="sp", bufs=3) as sp,
        ):
            for b in range(NBATCH):
                w = windows[b]
                halves = []
                for h in range(2):
                    it = sp.tile([128, NG * IW], mybir.dt.int16, tag="idx")
                    nc.sync.dma_start(out=it[:], in_=i_in[b, h])
                    g = gp.tile([128, LH * D], mybir.dt.float32, tag="gath")
                    for k in range(NG):
                        nc.gpsimd.dma_gather(
                            out_ap=g[:, k * 8 * D : (k + 1) * 8 * D].rearrange(
                                "p (t d) -> p t d", d=D
                            ),
                            in_ap=f_in[w : w + WINDOW, :],
                            idxs_ap=it[:, k * IW : (k + 1) * IW],
                            num_idxs=1024,
                            num_idxs_reg=1024,
                            elem_size=D,
                            queue_num=k % 4,
                            single_packet=False,
                        )
                    r = sp.tile([128, D], mybir.dt.float32, tag=f"red{h}")
                    nc.vector.tensor_reduce(
                        out=r[:],
                        in_=g[:].rearrange("p (t d) -> p d t", d=D),
                        axis=mybir.AxisListType.X,
                        op=mybir.AluOpType.max,
                    )
                    halves.append(r)
                o = sp.tile([128, D], mybir.dt.float32, tag="out")
                nc.vector.tensor_tensor(
                    out=o[:],
                    in0=halves[0][:],
                    in1=halves[1][:],
                    op=mybir.AluOpType.max,
                )
                nc.sync.dma_start(out=p_out[b * 128 : (b + 1) * 128, :], in_=o[:])

    if not nc.is_finalized():
        nc.finalize()
    return nc


def kernel(features, segment_ids, num_clusters):
    global _last_results
    from concourse.bass_utils import run_bass_kernel_spmd

    features = np.ascontiguousarray(np.asarray(features, dtype=np.float32))
    ids = np.asarray(segment_ids).astype(np.int64)
    nclusters = int(num_clusters)
    assert features.shape == (N_POINTS, D), features.shape
    assert ids.shape == (N_POINTS,)
    assert nclusters == N_CLUSTERS

    # --- host index prep -------------------------------------------------
    gstart = np.searchsorted(ids, np.arange(nclusters), side="left")
    gend = np.searchsorted(ids, np.arange(nclusters) + 1, side="left")
    gcounts = gend - gstart

    core_meta = []  # (cl_lo, ncl, s[NCL], cnt[NCL]) per core
    for c in range(N_CORES):
        r0, r1 = c * RPC, (c + 1) * RPC
        cl_lo, cl_hi = int(ids[r0]), int(ids[r1 - 1])
        ncl = cl_hi - cl_lo + 1
        assert ncl <= NCL, f"core {c}: {ncl} local clusters > {NCL}"
        s = np.clip(gstart[cl_lo : cl_hi + 1], r0, r1) - r0
        e = np.clip(gend[cl_lo : cl_hi + 1], r0, r1) - r0
        cnt = e - s
        s_pad = np.full(NCL, RPC - 1, dtype=np.int64)
        cnt_pad = np.zeros(NCL, dtype=np.int64)
        s_pad[:ncl] = s
        cnt_pad[:ncl] = cnt
        core_meta.append((cl_lo, ncl, s_pad, cnt_pad))

    L = max(int(m[3].max()) for m in core_meta)
    LH = (L + 1) // 2
    LH = max(8, (LH + 7) // 8 * 8)  # pad slots-per-half to a multiple of 8

    # Per-batch windows, shared across cores.
    windows = []
    for b in range(NBATCH):
        wmin = min(int(m[2][b * 128]) for m in core_meta)
        w = max(0, min(wmin, RPC - WINDOW))
        for m in core_meta:
            s_pad, cnt_pad = m[2], m[3]
            jj = slice(b * 128, (b + 1) * 128)
            last = s_pad[jj] + np.maximum(cnt_pad[jj] - 1, 0)
            active = cnt_pad[jj] > 0
            assert not active.any() or (
                (s_pad[jj][active] >= w).all()
                and (last[active] < w + WINDOW).all()
            ), f"batch {b}: window overflow"
        windows.append(w)

    # Build int16 gather tables: gidx[core][b, h, 128, NIDX//16]
    slots = np.arange(2 * LH)
    gidx_all = []
    for c in range(N_CORES):
        _, _, s_pad, cnt_pad = core_meta[c]
        # rows[j, s] = s_j + min(s, cnt_j - 1)   (cnt 0 -> window base)
        rows = s_pad[:, None] + np.minimum(slots[None, :], np.maximum(cnt_pad - 1, 0)[:, None])
        offs = np.empty((NBATCH, 128, 2 * LH), dtype=np.int64)
        for b in range(NBATCH):
            o = rows[b * 128 : (b + 1) * 128] - windows[b]
            o[cnt_pad[b * 128 : (b + 1) * 128] == 0] = 0
            offs[b] = o
        assert offs.min() >= 0 and offs.max() < WINDOW, (offs.min(), offs.max())
        offs16 = offs.astype(np.int16)
        # half h slots [h*LH,(h+1)*LH); gather j-index = t*128+p -> [j%16, j//16]
        NG = LH // 8
        g = np.empty((NBATCH, 2, 128, NG * 64), dtype=np.int16)
        for b in range(NBATCH):
            for h in range(2):
                A = offs16[b, :, h * LH : (h + 1) * LH]  # [128 p, LH]
                # gather k covers slots [k*8,(k+1)*8); flat j = t*128+p
                A = A.reshape(128, NG, 8).transpose(1, 2, 0).reshape(NG, 1024)
                W = A.reshape(NG, 64, 16).transpose(0, 2, 1)  # wrap [j%16, j//16]
                W = W.transpose(1, 0, 2).reshape(16, NG * 64)
                g[b, h] = np.tile(W, (8, 1))
        gidx_all.append(g)

    # --- build + run ------------------------------------------------------
    nc = _build_program(LH, windows)
    in_maps = [
        {
            "features": features[c * RPC : (c + 1) * RPC],
            "gidx": gidx_all[c],
        }
        for c in range(N_CORES)
    ]
    res = run_bass_kernel_spmd(nc, in_maps, list(range(N_CORES)))
    _last_results = res

    # --- host combine -----------------------------------------------------
    full = np.full((nclusters, D), -np.inf, dtype=np.float32)
    for c in range(N_CORES):
        cl_lo, ncl, _, _ = core_meta[c]
        part = res.results[c]["partial"][:ncl]
        full[cl_lo : cl_lo + ncl] = np.maximum(full[cl_lo : cl_lo + ncl], part)
    full[gcounts == 0] = -np.inf
    return full



# revision 2
# speedup vs baseline: 1.6443x; 1.6443x over previous
"""Trainium2 Bass kernel for sorted segment_max (ClusterPool).

Problem: features [2M, 128] f32, segment_ids [2M] sorted int, num_clusters
10000 -> output [10000, 128] = per-cluster max over rows.

Strategy (8 NeuronCores, SPMD):
  - Shard rows: core c gets rows [c*250k, (c+1)*250k). Sorted ids => each
    core covers a contiguous cluster range (~1252 clusters), padded to 1280
    local clusters = 10 batches x 128.
  - Gather granularity is an 8-row CHUNK (4 KB) instead of a single row
    (512 B): dma_gather with elem_size=1024 floats and elem_step=128 floats
    lets chunks start at arbitrary rows.  Cluster rows [s, e) are covered by
    chunks s, s+8, ..., with the last chunk anchored at e-8 (max is
    idempotent, so overlapping reads are harmless).  This cuts SWDGE
    descriptor-generation work 8x (the baseline bottleneck) and reduces
    padding (chunks pad to the per-batch max chunk count).
  - Device: per batch of 128 clusters (cluster p on partition p), NH gather
    calls of up to 8 chunks/cluster each; tensor_reduce max over rows of each
    gathered tile -> [128, 128]; tensor_tensor max-combine the partials;
    store to the partial output rows.
  - Clusters with fewer than 8 local rows (only possible at core-boundary
    splits) are computed on host and merged during the combine.
  - Host combines the 8 partial outputs with np.maximum.
"""

import os
import sys

import numpy as np

sys.path.insert(0, "/opt/trn_rl_repo")

N_POINTS = 2_000_000
D = 128
N_CLUSTERS = 10_000
N_CORES = 8
RPC = N_POINTS // N_CORES  # rows per core
NCL = 1280  # padded local clusters per core
NBATCH = NCL // 128
WINDOW = 32768  # dma_gather int16 index window (rows)
CH = 8  # rows per gather chunk (4 KB elements)
THMAX = 8  # max chunks per cluster per gather call

_last_results = None  # BassKernelResults of the most recent run (for test.py)


def _apply_drain_patch():
    """walrus TPB_CTRL supports a single sync wait; TileContext's tail drain
    accumulates one wait per outstanding proc.  Split them across NOPs."""
    import concourse.mybir as mybir
    import concourse.tile as tile
    from concourse.vector_clock import ScopedClock

    if getattr(tile.TileContext, "_drain_patched", False):
        return

    def _patched(self, tick_clock, wait_clock):
        nc = self.nc
        nop = nc.sync.nop(nofuse=True, hint="tail_drain_waits")
        wait_clock.add_sem_waits(nop.ins, ScopedClock({None: tick_clock.global_clock}))
        si = nop.ins.sync_info
        waits = list(si.on_wait) if si is not None and si.on_wait else []
        if len(waits) > 1:
            si.on_wait = waits[:1]
            for i in range(1, len(waits)):
                extra = nc.sync.nop(nofuse=True, hint=f"tail_drain_waits_{i}")
                if extra.ins.sync_info is None:
                    extra.ins.sync_info = mybir.SyncInfo(
                        on_wait=waits[i : i + 1], on_update=[]
                    )
                else:
                    extra.ins.sync_info.on_wait = waits[i : i + 1]
        nc.sync.drain()
        nc.all_engine_barrier()
        assert self.sems is not None
        popped = nc._tile_sem_poison_stack.pop()
        assert popped is self._sem_poison
        nc.clear_and_free_semaphores(list(self.sems.allocated().values()))
        nc.all_engine_barrier()

    tile.TileContext._drain_and_barrier = _patched
    tile.TileContext._drain_patched = True


def _build_program(tbs, windows):
    """Build the SPMD Bass program.

    tbs[b] = chunks per cluster for batch b (compile-time, shared by cores);
    windows[b] = window base row for batch b."""
    import concourse.bacc as bacc
    import concourse.mybir as mybir
    import concourse.tile as tile
    from concourse.bass import AP

    _apply_drain_patch()

    CMAX = max(tbs) * CH  # idx cols per batch table
    ELEM = CH * D  # floats per gather element

    nc = bacc.Bacc(None, num_swdge_queues=4)
    f_in = nc.dram_tensor("features", [RPC, D], mybir.dt.float32, kind="ExternalInput")
    i_in = nc.dram_tensor(
        "gidx", [NBATCH, 128, CMAX], mybir.dt.int16, kind="ExternalInput"
    )
    p_out = nc.dram_tensor(
        "partial", [NCL, D], mybir.dt.float32, kind="ExternalOutput"
    )

    fbase = f_in[:, :]
    gq = 0
    with tile.TileContext(nc) as tc:
        with (
            tc.tile_pool(name="gp", bufs=3) as gp,
            tc.tile_pool(name="ip", bufs=2) as ip,
            tc.tile_pool(name="sp", bufs=2) as sp,
        ):
            for b in range(NBATCH):
                T = tbs[b]
                w = windows[b]
                # overlapping window view: element = 8 rows (4 KB), step = 1 row
                win = AP(fbase.tensor, w * D, [[D, WINDOW], [1, ELEM]])
                it = ip.tile([128, CMAX], mybir.dt.int16, tag="idx")
                nc.sync.dma_start(out=it[:], in_=i_in[b])
                NH = (T + THMAX - 1) // THMAX
                acc = None
                for k in range(NH):
                    TH = min(THMAX, T - THMAX * k)
                    g = gp.tile([128, THMAX * ELEM], mybir.dt.float32, tag="gath")
                    nc.gpsimd.dma_gather(
                        out_ap=g[:, : TH * ELEM].rearrange(
                            "p (t e) -> p t e", e=ELEM
                        ),
                        in_ap=win,
                        idxs_ap=it[:, k * THMAX * CH : (k * THMAX + TH) * CH],
                        num_idxs=TH * 128,
                        num_idxs_reg=TH * 128,
                        elem_size=ELEM,
                        elem_step=D,
                        queue_num=gq % 4,
                        single_packet=False,
                    )
                    gq += 1
                    r = sp.tile([128, D], mybir.dt.float32, tag=f"red{k}")
                    nc.vector.tensor_reduce(
                        out=r[:],
                        in_=g[:, : TH * ELEM].rearrange("p (t d) -> p d t", d=D),
                        axis=mybir.AxisListType.X,
                        op=mybir.AluOpType.max,
                    )
                    if acc is None:
                        acc = r
                    else:
                        nc.vector.tensor_tensor(
                            out=acc[:], in0=acc[:], in1=r[:],
                            op=mybir.AluOpType.max,
                        )
                nc.sync.dma_start(out=p_out[b * 128 : (b + 1) * 128, :], in_=acc[:])

    if not nc.is_finalized():
        nc.finalize()
    return nc


def kernel(features, segment_ids, num_clusters):
    global _last_results
    from concourse.bass_utils import run_bass_kernel_spmd

    features = np.ascontiguousarray(np.asarray(features, dtype=np.float32))
    ids = np.asarray(segment_ids).astype(np.int64)
    nclusters = int(num_clusters)
    assert features.shape == (N_POINTS, D), features.shape
    assert ids.shape == (N_POINTS,)
    assert nclusters == N_CLUSTERS

    # --- host index prep -------------------------------------------------
    gstart = np.searchsorted(ids, np.arange(nclusters), side="left")
    gend = np.searchsorted(ids, np.arange(nclusters) + 1, side="left")
    gcounts = gend - gstart

    core_meta = []  # (cl_lo, ncl, s[NCL], cnt[NCL]) per core
    for c in range(N_CORES):
        r0, r1 = c * RPC, (c + 1) * RPC
        cl_lo, cl_hi = int(ids[r0]), int(ids[r1 - 1])
        ncl = cl_hi - cl_lo + 1
        assert ncl <= NCL, f"core {c}: {ncl} local clusters > {NCL}"
        s = np.clip(gstart[cl_lo : cl_hi + 1], r0, r1) - r0
        e = np.clip(gend[cl_lo : cl_hi + 1], r0, r1) - r0
        cnt = e - s
        s_pad = np.zeros(NCL, dtype=np.int64)
        cnt_pad = np.zeros(NCL, dtype=np.int64)
        s_pad[:ncl] = s
        cnt_pad[:ncl] = cnt
        core_meta.append((cl_lo, ncl, s_pad, cnt_pad))

    # Per-batch chunk counts and windows, shared across cores.
    nch_all = []  # per core: chunks needed per cluster (1 for inactive)
    for c in range(N_CORES):
        _, _, s_pad, cnt_pad = core_meta[c]
        active = cnt_pad >= CH
        nch = np.where(active, (cnt_pad + CH - 1) // CH, 1)
        nch_all.append(nch)

    tbs, windows = [], []
    for b in range(NBATCH):
        jj = slice(b * 128, (b + 1) * 128)
        T = max(int(m[jj].max()) for m in nch_all)
        tbs.append(T)
        wmin = RPC
        for c in range(N_CORES):
            _, _, s_pad, cnt_pad = core_meta[c]
            act = cnt_pad[jj] >= CH
            if act.any():
                wmin = min(wmin, int(s_pad[jj][act].min()))
        w = max(0, min(wmin, RPC - WINDOW - (CH - 1)))
        windows.append(w)

    CMAX = max(tbs) * CH

    # Build int16 chunk-start tables: gidx[core][b, 128, CMAX]
    gidx_all = []
    for c in range(N_CORES):
        _, _, s_pad, cnt_pad = core_meta[c]
        e_pad = s_pad + cnt_pad
        nch = nch_all[c]
        g = np.zeros((NBATCH, 128, CMAX), dtype=np.int16)
        for b in range(NBATCH):
            T = tbs[b]
            w = windows[b]
            jj = slice(b * 128, (b + 1) * 128)
            sj = s_pad[jj][:, None]
            ej = e_pad[jj][:, None]
            nj = nch[jj][:, None]
            act = (cnt_pad[jj] >= CH)[:, None]
            t = np.arange(T)[None, :]
            start = sj + CH * t
            start = np.where(t >= nj - 1, ej - CH, start)  # anchor last chunk
            start = np.where(act, start, w)  # dummy for small/empty clusters
            rel = start - w
            assert rel.min() >= 0 and rel.max() <= WINDOW - 1, (
                c, b, rel.min(), rel.max())
            assert int(np.where(act, start, 0).max()) + CH <= RPC
            V = rel.astype(np.int16).T.reshape(-1)  # j = t*128 + p
            tab = V.reshape(T * CH, 16).T  # wrap: [j%16, j//16]
            g[b, :, : T * CH] = np.tile(tab, (8, 1))
        gidx_all.append(g)

    # --- build + run ------------------------------------------------------
    nc = _build_program(tbs, windows)
    in_maps = [
        {
            "features": features[c * RPC : (c + 1) * RPC],
            "gidx": gidx_all[c],
        }
        for c in range(N_CORES)
    ]
    res = run_bass_kernel_spmd(nc, in_maps, list(range(N_CORES)))
    _last_results = res

    # --- host combine -----------------------------------------------------
    full = np.full((nclusters, D), -np.inf, dtype=np.float32)
    for c in range(N_CORES):
        cl_lo, ncl, s_pad, cnt_pad = core_meta[c]
        part = res.results[c]["partial"][:ncl]
        valid = cnt_pad[:ncl] >= CH
        rows = cl_lo + np.nonzero(valid)[0]
        full[rows] = np.maximum(full[rows], part[valid])
        # small boundary clusters: exact max on host
        small = np.nonzero((cnt_pad[:ncl] >= 1) & (cnt_pad[:ncl] < CH))[0]
        r0 = c * RPC
        for j in small:
            sj, cj = int(s_pad[j]), int(cnt_pad[j])
            mx = features[r0 + sj : r0 + sj + cj].max(axis=0)
            full[cl_lo + j] = np.maximum(full[cl_lo + j], mx)
    full[gcounts == 0] = -np.inf
    return full
